# revision 1
# baseline (speedup 1.0000x reference)
"""Trainium2 Bass kernel for the ConvE-style MoE-routing block.

Computes, for each batch row b:
    X = [e1|e2] @ rel_emb.T            # [B, NR] gating logits
    S, idx = top_k(sigmoid(X), 16)
    R1 = relu(rel_emb @ W_fcs.T + b)   # [NR, D]
    out = sum_k S_k * R1[idx_k] / sum_k S_k

Reformulated gather-free: zap the top-16 logits per row with two
(max8 + match_replace) rounds, then M = sigmoid(X) - sigmoid(X_zapped)
is exactly the top-16 sigmoid weights (0 elsewhere), so
    out = (M @ R1) / rowsum(M)
runs on the tensor engine as a dense matmul.

Precision: the gating matmul is fp32 (top-k selection-grade); R1 and
the combine matmul are float32r/fp16 (value-grade). M is stored fp16 so
its transpose rides the DMA xbar instead of the PE.

Layouts: the PE contracts along partitions, so the contraction operands
(stacked^T, R^T, W^T) are prepared host-side in numpy — pure input
marshalling, no FLOPs — and DMA'd directly; the kernel spends no engine
time on transposes except M^T (data-dependent, via DMA xbar).

Data-parallel over batch across 8 cores; rel_emb/W_fcs replicated;
R1 computation sharded across cores and AllGathered.
"""
import numpy as np

import concourse.bacc as bacc
import concourse.mybir as mybir
from concourse.bass_utils import run_bass_kernel_spmd
from concourse.tile import TileContext

P = 128
D = 512
TWO_D = 1024
NR = 2048
B = 8192
N_CORES = 8
BC = B // N_CORES      # 1024 batch rows per core
RT = BC // P           # 8 row tiles per core
KC = TWO_D // P        # 8 feature (contraction) chunks
NRC = NR // P          # 16 rel chunks
NLOC = NRC // N_CORES  # rel chunks per core for sharded R1
NEG = -60.0            # sigmoid(anything <= NEG + max|x|) == 0 to fp32

F32 = mybir.dt.float32
F32R = mybir.dt.float32r
F16 = mybir.dt.float16
AF = mybir.ActivationFunctionType

_CACHED = None


def _build():
    nc = bacc.Bacc("TRN2", target_bir_lowering=False, debug=True)
    # Host-transposed operand layouts (see module docstring).
    stT_d = nc.declare_dram_parameter("stackedT", [TWO_D, BC], F32, isOutput=False)
    relT = nc.declare_dram_parameter("rel_T", [TWO_D, NR], F32, isOutput=False)
    relsT = nc.declare_dram_parameter(
        "rel_sliceT", [TWO_D, NLOC * P], F32R, isOutput=False)
    wT = nc.declare_dram_parameter("W_T", [TWO_D, D], F32R, isOutput=False)
    bf = nc.declare_dram_parameter("b_fcs", [1, D], F32R, isOutput=False)
    out = nc.declare_dram_parameter("out", [BC, D], F32, isOutput=True)

    with TileContext(nc) as tc:
        with (
            tc.tile_pool(name="consts", bufs=1) as consts,
            tc.tile_pool(name="persist", bufs=1) as persist,
            tc.tile_pool(name="psx", bufs=3, space="PSUM") as psx,
            tc.tile_pool(name="pso", bufs=2, space="PSUM") as pso,
        ):
            ones1_f32 = consts.tile([1, P], F32)
            nc.vector.memset(ones1_f32, 1.0)
            ones1 = consts.tile([1, P], F32R)
            nc.vector.tensor_copy(ones1, ones1_f32)
            b_sb = consts.tile([1, D], F32R)
            nc.sync.dma_start(out=b_sb, in_=bf[:])

            # R^T feature-chunks as separate tiles (fine-grained deps: the
            # gating k-step only waits for its own chunk's DMA), loads
            # spread over both HW-DGE rings.
            rt_k = []
            for k in range(KC):
                t = persist.tile([P, NR], F32, tag=f"rt{k}")
                (nc.sync if k % 2 == 0 else nc.scalar).dma_start(
                    out=t, in_=relT[k * P:(k + 1) * P, :])
                rt_k.append(t)
            # W^T: feature-chunk k at cols [k*D, (k+1)*D), f32r for R1.
            wt_sb = persist.tile([P, KC * D], F32R)
            for k in range(KC):
                nc.scalar.dma_start(
                    out=wt_sb[:, k * D:(k + 1) * D],
                    in_=wT[k * P:(k + 1) * P, :])
            # This core's R^T slice for the sharded R1 (f32r lhsT).
            rstage = persist.tile([P, KC * NLOC * P], F32R)
            for k in range(KC):
                nc.scalar.dma_start(
                    out=rstage[:, k * NLOC * P:(k + 1) * NLOC * P],
                    in_=relsT[k * P:(k + 1) * P, :])
            # R1: rel-chunk c at cols [c*D, (c+1)*D), fp16 (value-grade).
            r1_sb = persist.tile([P, NRC * D], F16)

            # Sharded R1 = relu(R @ W^T + b): 2 chunks here, AllGather the
            # rest while the PE starts on the gating tiles.
            with tc.tile_pool(name="dram", bufs=1, space="DRAM") as dram:
                r1_loc = persist.tile([P, NLOC * D], F16)
                for cl in range(NLOC):
                    pr = pso.tile([P, D], F32, tag="pso")
                    for k in range(KC):
                        nc.tensor.matmul(
                            pr,
                            lhsT=rstage[:, (k * NLOC + cl) * P:
                                        (k * NLOC + cl + 1) * P],
                            rhs=wt_sb[:, k * D:(k + 1) * D],
                            start=(k == 0),
                            stop=False,
                        )
                    nc.tensor.matmul(
                        pr, lhsT=ones1, rhs=b_sb, start=False, stop=True)
                    nc.scalar.activation(
                        r1_loc[:, cl * D:(cl + 1) * D], pr, AF.Relu)
                r1_loc_dram = dram.tile([P, NLOC * D], F16)
                nc.sync.dma_start(out=r1_loc_dram[:], in_=r1_loc)
                r1_ag = dram.tile([N_CORES * P, NLOC * D], F16)
                nc.gpsimd.collective_compute(
                    "AllGather",
                    mybir.AluOpType.bypass,
                    replica_groups=[list(range(N_CORES))],
                    ins=[r1_loc_dram.opt()],
                    outs=[r1_ag.opt()],
                )
                # Readbacks wait on the AllGather — keep them on the idle
                # gpsimd queue so they don't block other DMA traffic.
                for j in range(N_CORES):
                    for cl in range(NLOC):
                        c = j * NLOC + cl
                        nc.gpsimd.dma_start(
                            out=r1_sb[:, c * D:(c + 1) * D],
                            in_=r1_ag[j * P:(j + 1) * P, cl * D:(cl + 1) * D],
                        )

                with tc.tile_pool(name="work", bufs=2) as work:
                    # Software pipeline: tile m's combine work runs after
                    # tile m+1's gating so the PE never waits in FIFO order
                    # on the serial DVE top-k chain.
                    pending = None

                    def combine_phase(mm, mf, rec):
                        # M^T via one xbar DMA: out[p, c, j] = in[j, c*P+p].
                        mt = work.tile([P, NRC * P], F16, tag="mt")
                        nc.sync.dma_start_transpose(
                            mt[:].rearrange("p (c j) -> p c j", c=NRC), mf)
                        op = pso.tile([P, D], F32, tag="pso")
                        for c in range(NRC):
                            nc.tensor.matmul(
                                op,
                                lhsT=mt[:, c * P:(c + 1) * P],
                                rhs=r1_sb[:, c * D:(c + 1) * D],
                                start=(c == 0),
                                stop=(c == NRC - 1),
                            )
                        ot = work.tile([P, D], F32, tag="ot")
                        nc.scalar.activation(ot, op, AF.Copy, scale=rec)
                        nc.sync.dma_start(
                            out=out[mm * P:(mm + 1) * P, :], in_=ot)

                    for m in range(RT):
                        # stacked^T row-tile: feature-chunk k at cols
                        # [k*P, (k+1)*P); one strided DMA from host layout.
                        stt = work.tile([P, TWO_D], F32, tag="stt")
                        for k in range(KC):
                            nc.sync.dma_start(
                                out=stt[:, k * P:(k + 1) * P],
                                in_=stT_d[k * P:(k + 1) * P,
                                          m * P:(m + 1) * P],
                            )

                        # Gating X = stacked @ R^T, fp32 (selection-grade).
                        # k-outer so each stationary loads once per tile
                        # (4 consecutive MMs per LDWEIGHTS).
                        xs = work.tile([P, NR], F32, tag="xs")
                        xp0 = psx.tile([P, TWO_D], F32, tag="xph")
                        xp1 = psx.tile([P, TWO_D], F32, tag="xph")
                        xp = (xp0, xp1)
                        for k in range(KC):
                            for hb in range(2):
                                for nb in range(2):
                                    nc.tensor.matmul(
                                        xp[hb][:, nb * 512:(nb + 1) * 512],
                                        lhsT=stt[:, k * P:(k + 1) * P],
                                        rhs=rt_k[k][:, (hb * 2 + nb) * 512:
                                                     (hb * 2 + nb + 1) * 512],
                                        start=(k == 0),
                                        stop=(k == KC - 1),
                                    )
                        for q in range(4):
                            nc.scalar.activation(
                                xs[:, q * 512:(q + 1) * 512],
                                xp[q // 2][:, (q % 2) * 512:(q % 2 + 1) * 512],
                                AF.Copy)

                        # Zap top-16 values.
                        m1 = work.tile([P, 8], F32, tag="m1")
                        nc.vector.max(out=m1, in_=xs)
                        xz = work.tile([P, NR], F32, tag="xz")
                        nc.vector.match_replace(
                            out=xz, in_to_replace=m1, in_values=xs,
                            imm_value=NEG)
                        m2 = work.tile([P, 8], F32, tag="m2")
                        nc.vector.max(out=m2, in_=xz)
                        nc.vector.match_replace(
                            out=xz, in_to_replace=m2, in_values=xz,
                            imm_value=NEG)

                        # M = sigmoid(X) - sigmoid(X_zapped), fp16 (the
                        # non-selected entries are identical fp16 values in
                        # both sigmoids and cancel exactly); denom via the
                        # activation accumulators.
                        s_all = work.tile([P, NR], F16, tag="s_all")
                        acc_all = work.tile([P, 1], F32, tag="acc_all")
                        nc.scalar.activation(
                            s_all, xs, AF.Sigmoid, accum_out=acc_all)
                        s_exc = work.tile([P, NR], F16, tag="s_exc")
                        acc_exc = work.tile([P, 1], F32, tag="acc_exc")
                        nc.scalar.activation(
                            s_exc, xz, AF.Sigmoid, accum_out=acc_exc)
                        mf = work.tile([P, NR], F16, tag="mf")
                        nc.vector.tensor_sub(mf, s_all, s_exc)
                        den = work.tile([P, 1], F32, tag="den")
                        nc.vector.tensor_sub(den, acc_all, acc_exc)
                        rec = work.tile([P, 1], F32, tag="rec")
                        nc.vector.reciprocal(rec, den)

                        if pending is not None:
                            combine_phase(*pending)
                        pending = (m, mf, rec)
                    combine_phase(*pending)

    nc.finalize()
    return nc


def _get_nc():
    global _CACHED
    if _CACHED is None:
        _CACHED = _build()
    return _CACHED


def _make_in_maps(e1, e2, rel_emb, W_fcs, b_fcs):
    e1 = np.asarray(e1, dtype=np.float32)
    e2 = np.asarray(e2, dtype=np.float32)
    rel_emb = np.asarray(rel_emb, dtype=np.float32)
    W_fcs = np.asarray(W_fcs, dtype=np.float32)
    b_fcs = np.asarray(b_fcs, dtype=np.float32).reshape(1, D)

    stackedT = np.ascontiguousarray(
        np.concatenate([e1, e2], axis=1).T)          # [2D, B]
    rel_T = np.ascontiguousarray(rel_emb.T)          # [2D, NR]
    W_T = np.ascontiguousarray(W_fcs.T)              # [2D, D]
    nsl = NLOC * P
    return [
        {
            "stackedT": np.ascontiguousarray(
                stackedT[:, c * BC:(c + 1) * BC]),
            "rel_T": rel_T,
            "rel_sliceT": np.ascontiguousarray(
                rel_emb[c * nsl:(c + 1) * nsl].T),
            "W_T": W_T,
            "b_fcs": b_fcs,
        }
        for c in range(N_CORES)
    ]


def kernel(e1, e2, rel_emb, W_fcs, b_fcs, **_ignored):
    nc = _get_nc()
    in_maps = _make_in_maps(e1, e2, rel_emb, W_fcs, b_fcs)
    res = run_bass_kernel_spmd(nc, in_maps, list(range(N_CORES)))
    return np.concatenate(
        [res.results[c]["out"] for c in range(N_CORES)], axis=0)



# revision 8
# speedup vs baseline: 1.0176x; 1.0176x over previous
"""Trainium2 Bass kernel for the ConvE-style MoE-routing block.

Computes, for each batch row b:
    X = [e1|e2] @ rel_emb.T            # [B, NR] gating logits
    S, idx = top_k(sigmoid(X), 16)
    R1 = relu(rel_emb @ W_fcs.T + b)   # [NR, D]
    out = sum_k S_k * R1[idx_k] / sum_k S_k

Reformulated gather-free: zap the top-16 logits per row with two
(max8 + match_replace) rounds, then M = sigmoid(X) - sigmoid(X_zapped)
is exactly the top-16 sigmoid weights (0 elsewhere), so
    out = (M @ R1) / rowsum(M)
runs on the tensor engine as a dense matmul.

Precision: fp32 matmul costs 4 PE cycles/row and f32r (1 cycle/row)
truncates inputs to ~bf16, which flips ~25% of rows' top-16 sets. The
gating here instead uses a 3-term fp16 split at full PE rate:
    X*SA*SB = a_hi@b_hi + a_lo@b_hi + a_hi@b_lo
with a = stacked*SA, b = rel*SB pre-split hi/lo into fp16 host-side.
The scales keep every split value in fp16 normal range (subnormal-FTZ
safe); residual error ~2e-7 abs vs top-16 gap scale ~1.5e-2, giving 0
selection flips on N(0,1)-type data (validated offline vs fp64).
Sigmoids fold the 1/(SA*SB) rescale into the ACT input scale. R1 and
the combine matmul are fp16 (value-grade).

Layouts: the PE contracts along partitions, so all contraction
operands are marshalled host-side in numpy (pure input marshalling, no
FLOPs) into SBUF-native [128, free] layouts and DMA'd with one
descriptor each; the kernel spends no engine time on transposes except
M^T (data-dependent, via DMA xbar).

Data-parallel over batch across 8 cores; rel/W replicated; R1 sharded
across cores and AllGathered, hidden behind a 3-tile-deep gating/
combine software pipeline. DMA traffic is spread over the three
DMA-capable queues (sync/scalar/gpsimd) ordered by first use: tile-0
stacked splits lead the sync ring so the PE starts within ~2us, then
rel_hi; rel_lo + R1 operands ride scalar/gpsimd; output stores ride
gpsimd so they never block the stacked-tile prefetches.
"""
import numpy as np

import concourse.bacc as bacc
import concourse.mybir as mybir
from concourse.bass_utils import run_bass_kernel_spmd
from concourse.tile import TileContext

P = 128
D = 512
TWO_D = 1024
NR = 2048
B = 8192
N_CORES = 8
BC = B // N_CORES      # 1024 batch rows per core
RT = BC // P           # 8 row tiles per core
KC = TWO_D // P        # 8 feature (contraction) chunks
NRC = NR // P          # 16 rel chunks
NLOC = NRC // N_CORES  # rel chunks per core for sharded R1
PEND = 3               # combine pipeline depth (hides the AllGather)

SA = 64.0              # stacked pre-scale (fp16-normal-range splits)
SB = 256.0             # rel pre-scale
ISCALE = 1.0 / (SA * SB)
NEG = -1.1e6           # sigmoid(NEG*ISCALE) ~ 8e-30: cancels exactly

F32 = mybir.dt.float32
F16 = mybir.dt.float16
AF = mybir.ActivationFunctionType

_CACHED = None


def _build():
    nc = bacc.Bacc("TRN2", target_bir_lowering=False, debug=True)
    # Host-marshalled operand layouts (see module docstring).
    a_hi_d = nc.declare_dram_parameter("A_hi", [RT * P, TWO_D], F16, isOutput=False)
    a_lo_d = nc.declare_dram_parameter("A_lo", [RT * P, TWO_D], F16, isOutput=False)
    rH = nc.declare_dram_parameter("relT_hi", [TWO_D, NR], F16, isOutput=False)
    rL = nc.declare_dram_parameter("relT_lo", [TWO_D, NR], F16, isOutput=False)
    rsl = nc.declare_dram_parameter("rel_slice_k", [P, KC * NLOC * P], F16,
                                    isOutput=False)
    wk = nc.declare_dram_parameter("W_k", [P, KC * D], F16, isOutput=False)
    bf = nc.declare_dram_parameter("b_fcs", [1, D], F16, isOutput=False)
    out = nc.declare_dram_parameter("out", [BC, D], F32, isOutput=True)

    with TileContext(nc) as tc:
        with (
            tc.tile_pool(name="consts", bufs=1) as consts,
            tc.tile_pool(name="persist", bufs=1) as persist,
            tc.tile_pool(name="psx", bufs=3, space="PSUM") as psx,
            tc.tile_pool(name="pso", bufs=2, space="PSUM") as pso,
            tc.tile_pool(name="dram", bufs=1, space="DRAM") as dram,
            tc.tile_pool(name="work", bufs=2) as work,
            tc.tile_pool(name="comb", bufs=PEND + 1) as comb,
        ):
            ones1_f32 = consts.tile([1, P], F32)
            nc.vector.memset(ones1_f32, 1.0)
            ones1 = consts.tile([1, P], F16)
            nc.vector.tensor_copy(ones1, ones1_f32)

            # Tile-0 stacked splits lead the sync ring: the PE's first
            # gating matmul depends only on these + bh_0.
            a_tiles = {}

            def load_a(m):
                ah = work.tile([P, TWO_D], F16, tag="ah")
                nc.sync.dma_start(out=ah, in_=a_hi_d[m * P:(m + 1) * P, :])
                al = work.tile([P, TWO_D], F16, tag="al")
                nc.sync.dma_start(out=al, in_=a_lo_d[m * P:(m + 1) * P, :])
                a_tiles[m] = (ah, al)

            load_a(0)

            # Gating rel chunks: b_hi next on the sync ring; b_lo split
            # over the scalar/gpsimd rings (needed only from the third
            # gating pass of each tile).
            bh_k = []
            bl_k = []
            for k in range(KC):
                t = persist.tile([P, NR], F16, tag=f"bh{k}")
                nc.sync.dma_start(out=t, in_=rH[k * P:(k + 1) * P, :])
                bh_k.append(t)
            # R1 operands lead the scalar ring so the sharded R1 (and its
            # AllGather) launches right after gating tile 0.
            rstage = persist.tile([P, KC * NLOC * P], F16)
            nc.scalar.dma_start(out=rstage, in_=rsl[:])
            wt_sb = persist.tile([P, KC * D], F16)
            nc.scalar.dma_start(out=wt_sb, in_=wk[:])
            b_sb = consts.tile([1, D], F16)
            nc.scalar.dma_start(out=b_sb, in_=bf[:])
            for k in range(KC):
                t = persist.tile([P, NR], F16, tag=f"bl{k}")
                (nc.scalar if k % 2 == 0 else nc.gpsimd).dma_start(
                    out=t, in_=rL[k * P:(k + 1) * P, :])
                bl_k.append(t)

            # R1: rel-chunk c at cols [c*D, (c+1)*D), fp16 (value-grade).
            r1_sb = persist.tile([P, NRC * D], F16)

            def gating_phase(m):
                if m + 1 < RT:
                    load_a(m + 1)   # prefetch behind this tile's compute
                ah, al = a_tiles.pop(m)

                # Gating X*SA*SB via the 3-term fp16 split, fp32 PSUM
                # accumulation (24 matmuls per 512-col bank). Term order
                # matches DMA arrival: hi@hi, lo@hi (rel_hi again), hi@lo.
                xs = work.tile([P, NR], F32, tag="xs")
                xp0 = psx.tile([P, TWO_D], F32, tag="xph")
                xp1 = psx.tile([P, TWO_D], F32, tag="xph")
                xp = (xp0, xp1)
                for t, (A, Bk) in enumerate(
                        ((ah, bh_k), (al, bh_k), (ah, bl_k))):
                    for k in range(KC):
                        lhs = A[:, k * P:(k + 1) * P]
                        for hb in range(2):
                            for nb in range(2):
                                nc.tensor.matmul(
                                    xp[hb][:, nb * 512:(nb + 1) * 512],
                                    lhsT=lhs,
                                    rhs=Bk[k][:, (hb * 2 + nb) * 512:
                                              (hb * 2 + nb + 1) * 512],
                                    start=(t == 0 and k == 0),
                                    stop=(t == 2 and k == KC - 1),
                                )
                for q in range(4):
                    nc.scalar.activation(
                        xs[:, q * 512:(q + 1) * 512],
                        xp[q // 2][:, (q % 2) * 512:(q % 2 + 1) * 512],
                        AF.Copy)

                # Zap top-16 values (fp32 scan, selection-grade).
                m1 = work.tile([P, 8], F32, tag="m1")
                nc.vector.max(out=m1, in_=xs)
                xz = work.tile([P, NR], F32, tag="xz")
                nc.vector.match_replace(
                    out=xz, in_to_replace=m1, in_values=xs, imm_value=NEG)
                m2 = work.tile([P, 8], F32, tag="m2")
                nc.vector.max(out=m2, in_=xz)
                nc.vector.match_replace(
                    out=xz, in_to_replace=m2, in_values=xz, imm_value=NEG)

                # M = sigmoid(X) - sigmoid(X_zapped), fp16 (the
                # non-selected entries are identical fp16 values in both
                # sigmoids and cancel exactly); the 1/(SA*SB) rescale
                # rides the ACT input scale; denom via the activation
                # accumulators.
                s_all = work.tile([P, NR], F16, tag="s_all")
                acc_all = work.tile([P, 1], F32, tag="acc_all")
                nc.scalar.activation(
                    s_all, xs, AF.Sigmoid, scale=ISCALE, accum_out=acc_all)
                s_exc = work.tile([P, NR], F16, tag="s_exc")
                acc_exc = work.tile([P, 1], F32, tag="acc_exc")
                nc.scalar.activation(
                    s_exc, xz, AF.Sigmoid, scale=ISCALE, accum_out=acc_exc)
                mf = comb.tile([P, NR], F16, tag="mf")
                nc.vector.tensor_sub(mf, s_all, s_exc)
                den = work.tile([P, 1], F32, tag="den")
                nc.vector.tensor_sub(den, acc_all, acc_exc)
                rec = comb.tile([P, 1], F32, tag="rec")
                nc.vector.reciprocal(rec, den)
                return mf, rec

            def combine_phase(mm, mf, rec):
                # M^T via one xbar DMA: out[p, c, j] = in[j, c*P+p].
                mt = comb.tile([P, NRC * P], F16, tag="mt")
                nc.sync.dma_start_transpose(
                    mt[:].rearrange("p (c j) -> p c j", c=NRC), mf)
                op = pso.tile([P, D], F32, tag="pso")
                for c in range(NRC):
                    nc.tensor.matmul(
                        op,
                        lhsT=mt[:, c * P:(c + 1) * P],
                        rhs=r1_sb[:, c * D:(c + 1) * D],
                        start=(c == 0),
                        stop=(c == NRC - 1),
                    )
                ot = work.tile([P, D], F32, tag="ot")
                nc.scalar.activation(ot, op, AF.Copy, scale=rec)
                # Output stores on gpsimd: they trail the combine and must
                # not block the sync ring's stacked-tile prefetches.
                nc.gpsimd.dma_start(
                    out=out[mm * P:(mm + 1) * P, :], in_=ot)

            # Software pipeline: gating tile 0 first (PE starts ~2us in),
            # then the sharded R1 + AllGather launch under it; tile m's
            # combine runs after tile m+PEND-1's gating so the PE never
            # waits on the serial DVE top-k chain or the AllGather.
            pending = [(0, *gating_phase(0))]

            # Sharded R1 = relu(R @ W^T + b): 2 chunks here, AllGather
            # the rest while the PE works through the gating tiles.
            r1_loc = persist.tile([P, NLOC * D], F16)
            for cl in range(NLOC):
                pr = pso.tile([P, D], F32, tag="pso")
                for k in range(KC):
                    nc.tensor.matmul(
                        pr,
                        lhsT=rstage[:, (k * NLOC + cl) * P:
                                    (k * NLOC + cl + 1) * P],
                        rhs=wt_sb[:, k * D:(k + 1) * D],
                        start=(k == 0),
                        stop=False,
                    )
                nc.tensor.matmul(
                    pr, lhsT=ones1, rhs=b_sb, start=False, stop=True)
                nc.scalar.activation(
                    r1_loc[:, cl * D:(cl + 1) * D], pr, AF.Relu)
            r1_loc_dram = dram.tile([P, NLOC * D], F16)
            nc.sync.dma_start(out=r1_loc_dram[:], in_=r1_loc)
            r1_ag = dram.tile([N_CORES * P, NLOC * D], F16)
            nc.gpsimd.collective_compute(
                "AllGather",
                mybir.AluOpType.bypass,
                replica_groups=[list(range(N_CORES))],
                ins=[r1_loc_dram.opt()],
                outs=[r1_ag.opt()],
            )
            # Readbacks wait on the AllGather — keep them on the gpsimd
            # queue (its bl loads are done early) so they don't block
            # other DMA traffic.
            for j in range(N_CORES):
                for cl in range(NLOC):
                    c = j * NLOC + cl
                    nc.gpsimd.dma_start(
                        out=r1_sb[:, c * D:(c + 1) * D],
                        in_=r1_ag[j * P:(j + 1) * P, cl * D:(cl + 1) * D],
                    )

            for m in range(1, RT):
                pending.append((m, *gating_phase(m)))
                if len(pending) >= PEND:
                    combine_phase(*pending.pop(0))
            while pending:
                combine_phase(*pending.pop(0))

    nc.finalize()
    return nc


def _get_nc():
    global _CACHED
    if _CACHED is None:
        _CACHED = _build()
    return _CACHED


def _split16(x):
    hi = x.astype(np.float16)
    lo = (x - hi.astype(np.float32)).astype(np.float16)
    return hi, lo


def _chunk_part(x):
    """[TWO_D, N] -> [P, KC*N]: feature-chunk k at cols [k*N, (k+1)*N)."""
    n = x.shape[1]
    return np.ascontiguousarray(
        x.reshape(KC, P, n).transpose(1, 0, 2).reshape(P, KC * n))


def _make_in_maps(e1, e2, rel_emb, W_fcs, b_fcs):
    e1 = np.asarray(e1, dtype=np.float32)
    e2 = np.asarray(e2, dtype=np.float32)
    rel_emb = np.asarray(rel_emb, dtype=np.float32)
    W_fcs = np.asarray(W_fcs, dtype=np.float32)
    b_fcs = np.asarray(b_fcs, dtype=np.float32).reshape(1, D)

    stacked = np.concatenate([e1, e2], axis=1) * SA   # [B, 2D]
    a_hi, a_lo = _split16(stacked)
    relT = np.ascontiguousarray((rel_emb * SB).T)     # [2D, NR]
    r_hi, r_lo = _split16(relT)

    # A tiles: [RT*P, TWO_D] with A[m*P+p, k*P+j] = stacked[m*P+j, k*P+p]
    def a_tiles(a):
        return np.ascontiguousarray(
            a.reshape(RT, P, KC, P).transpose(0, 3, 2, 1).reshape(RT * P, TWO_D))

    wkm = _chunk_part(np.ascontiguousarray(W_fcs.T)).astype(np.float16)
    nsl = NLOC * P
    return [
        {
            "A_hi": a_tiles(a_hi[c * BC:(c + 1) * BC]),
            "A_lo": a_tiles(a_lo[c * BC:(c + 1) * BC]),
            "relT_hi": r_hi,
            "relT_lo": r_lo,
            "rel_slice_k": _chunk_part(
                np.ascontiguousarray(rel_emb[c * nsl:(c + 1) * nsl].T)
            ).astype(np.float16),
            "W_k": wkm,
            "b_fcs": b_fcs.astype(np.float16),
        }
        for c in range(N_CORES)
    ]


def kernel(e1, e2, rel_emb, W_fcs, b_fcs, **_ignored):
    nc = _get_nc()
    in_maps = _make_in_maps(e1, e2, rel_emb, W_fcs, b_fcs)
    res = run_bass_kernel_spmd(nc, in_maps, list(range(N_CORES)))
    return np.concatenate(
        [res.results[c]["out"] for c in range(N_CORES)], axis=0)


# revision 14
# speedup vs baseline: 1.3951x; 1.3710x over previous
"""Trainium2 Bass kernel for the ConvE-style MoE-routing block.

Computes, for each batch row b:
    X = [e1|e2] @ rel_emb.T            # [B, NR] gating logits
    S, idx = top_k(sigmoid(X), 16)
    R1 = relu(rel_emb @ W_fcs.T + b)   # [NR, D]
    out = sum_k S_k * R1[idx_k] / sum_k S_k

Reformulated gather-free: zap the top-16 logits per row with two
(max8 + match_replace) rounds, then M = sigmoid(X) - sigmoid(X_zapped)
is exactly the top-16 sigmoid weights (0 elsewhere), so
    out = (M @ R1) / rowsum(M)
runs on the tensor engine as a dense matmul.

Precision: fp32 matmul costs 4 PE cycles/row and f32r (1 cycle/row)
truncates inputs to ~bf16, which flips ~25% of rows' top-16 sets. The
gating here instead uses a 3-term fp16 split at full PE rate:
    X*SA*SB = a_hi@b_hi + a_lo@b_hi + a_hi@b_lo
with a = stacked*SA, b = rel*SB pre-split hi/lo into fp16 host-side.
The scales keep every split value in fp16 normal range (subnormal-FTZ
safe); residual error ~2e-7 abs vs top-16 gap scale ~1.5e-2, giving 0
selection flips on N(0,1)-type data (validated offline vs fp64).
Sigmoids fold the 1/(SA*SB) rescale into the ACT input scale. R1 and
the combine matmul are fp16 (value-grade).

Layouts: the PE contracts along partitions, so all contraction
operands are marshalled host-side in numpy (pure input marshalling, no
FLOPs) into SBUF-native [128, free] layouts and DMA'd with one
descriptor each; the kernel spends no engine time on transposes except
M^T (data-dependent, via DMA xbar).

Data-parallel over batch across 8 cores; rel/W replicated; R1 computed
fully locally on every core straight from the already-loaded scaled
rel_hi chunks (an AllGather of a sharded R1 measured ~90us of
collective latency and stalled the in-order PE queue — recomputing is
144 matmuls ~= 38us and makes the cores fully independent). Combines
trail gating by PEND tiles so the PE never waits on the serial DVE
top-k chain. DMA traffic is spread over the three DMA-capable queues
(sync/scalar/gpsimd); output stores ride gpsimd so they never block
the stacked-tile prefetches.
"""
import numpy as np

import concourse.bacc as bacc
import concourse.mybir as mybir
from concourse.bass_utils import run_bass_kernel_spmd
from concourse.tile import TileContext

P = 128
D = 512
TWO_D = 1024
NR = 2048
B = 8192
N_CORES = 8
BC = B // N_CORES      # 1024 batch rows per core
RT = BC // P           # 8 row tiles per core
KC = TWO_D // P        # 8 feature (contraction) chunks
NRC = NR // P          # 16 rel chunks
NLOC = NRC // N_CORES  # rel chunks per core for sharded R1
PEND = 3               # combine pipeline depth (hides the AllGather)

SA = 64.0              # stacked pre-scale (fp16-normal-range splits)
SB = 256.0             # rel pre-scale
ISCALE = 1.0 / (SA * SB)
NEG = -1.1e6           # sigmoid(NEG*ISCALE) ~ 8e-30: cancels exactly

F32 = mybir.dt.float32
F16 = mybir.dt.float16
AF = mybir.ActivationFunctionType

_CACHED = None


def _build():
    nc = bacc.Bacc("TRN2", target_bir_lowering=False, debug=True)
    # Host-marshalled operand layouts (see module docstring).
    a_hi_d = nc.declare_dram_parameter("A_hi", [RT * P, TWO_D], F16, isOutput=False)
    a_lo_d = nc.declare_dram_parameter("A_lo", [RT * P, TWO_D], F16, isOutput=False)
    rH = nc.declare_dram_parameter("relT_hi", [TWO_D, NR], F16, isOutput=False)
    rL = nc.declare_dram_parameter("relT_lo", [TWO_D, NR], F16, isOutput=False)
    wk = nc.declare_dram_parameter("W_k", [P, KC * D], F16, isOutput=False)
    bf = nc.declare_dram_parameter("b_fcs", [1, D], F16, isOutput=False)
    out = nc.declare_dram_parameter("out", [BC, D], F32, isOutput=True)

    with TileContext(nc) as tc:
        with (
            tc.tile_pool(name="consts", bufs=1) as consts,
            tc.tile_pool(name="persist", bufs=1) as persist,
            tc.tile_pool(name="psx", bufs=3, space="PSUM") as psx,
            tc.tile_pool(name="pso", bufs=2, space="PSUM") as pso,
            tc.tile_pool(name="work", bufs=2) as work,
            tc.tile_pool(name="comb", bufs=PEND + 1) as comb,
        ):
            ones1_f32 = consts.tile([1, P], F32)
            nc.vector.memset(ones1_f32, 1.0)
            ones1 = consts.tile([1, P], F16)
            nc.vector.tensor_copy(ones1, ones1_f32)

            # Tile-0 stacked splits lead the sync ring: the PE's first
            # gating matmul depends only on these + bh_0.
            a_tiles = {}

            def load_a(m):
                ah = work.tile([P, TWO_D], F16, tag="ah")
                nc.sync.dma_start(out=ah, in_=a_hi_d[m * P:(m + 1) * P, :])
                al = work.tile([P, TWO_D], F16, tag="al")
                nc.sync.dma_start(out=al, in_=a_lo_d[m * P:(m + 1) * P, :])
                a_tiles[m] = (ah, al)

            load_a(0)

            # Gating rel chunks: b_hi next on the sync ring; b_lo split
            # over the scalar/gpsimd rings (needed only from the third
            # gating pass of each tile).
            bh_k = []
            bl_k = []
            for k in range(KC):
                t = persist.tile([P, NR], F16, tag=f"bh{k}")
                nc.sync.dma_start(out=t, in_=rH[k * P:(k + 1) * P, :])
                bh_k.append(t)
            # R1 operands lead the scalar ring so R1 can run right after
            # gating tile 0.
            wt_sb = persist.tile([P, KC * D], F16)
            nc.scalar.dma_start(out=wt_sb, in_=wk[:])
            b_sb = consts.tile([1, D], F16)
            nc.scalar.dma_start(out=b_sb, in_=bf[:])
            for k in range(KC):
                t = persist.tile([P, NR], F16, tag=f"bl{k}")
                (nc.scalar if k % 2 == 0 else nc.gpsimd).dma_start(
                    out=t, in_=rL[k * P:(k + 1) * P, :])
                bl_k.append(t)

            # R1: rel-chunk c at cols [c*D, (c+1)*D), fp16 (value-grade).
            r1_sb = persist.tile([P, NRC * D], F16)

            def gating_phase(m):
                if m + 1 < RT:
                    load_a(m + 1)   # prefetch behind this tile's compute
                ah, al = a_tiles.pop(m)

                # Gating X*SA*SB via the 3-term fp16 split, fp32 PSUM
                # accumulation (24 matmuls per 512-col bank). Term order
                # matches DMA arrival: hi@hi, lo@hi (rel_hi again), hi@lo.
                xs = work.tile([P, NR], F32, tag="xs")
                xp0 = psx.tile([P, TWO_D], F32, tag="xph")
                xp1 = psx.tile([P, TWO_D], F32, tag="xph")
                xp = (xp0, xp1)
                for t, (A, Bk) in enumerate(
                        ((ah, bh_k), (al, bh_k), (ah, bl_k))):
                    for k in range(KC):
                        lhs = A[:, k * P:(k + 1) * P]
                        for hb in range(2):
                            for nb in range(2):
                                nc.tensor.matmul(
                                    xp[hb][:, nb * 512:(nb + 1) * 512],
                                    lhsT=lhs,
                                    rhs=Bk[k][:, (hb * 2 + nb) * 512:
                                              (hb * 2 + nb + 1) * 512],
                                    start=(t == 0 and k == 0),
                                    stop=(t == 2 and k == KC - 1),
                                )
                for q in range(4):
                    nc.scalar.activation(
                        xs[:, q * 512:(q + 1) * 512],
                        xp[q // 2][:, (q % 2) * 512:(q % 2 + 1) * 512],
                        AF.Copy)

                # Zap top-16 values (fp32 scan, selection-grade).
                m1 = work.tile([P, 8], F32, tag="m1")
                nc.vector.max(out=m1, in_=xs)
                xz = work.tile([P, NR], F32, tag="xz")
                nc.vector.match_replace(
                    out=xz, in_to_replace=m1, in_values=xs, imm_value=NEG)
                m2 = work.tile([P, 8], F32, tag="m2")
                nc.vector.max(out=m2, in_=xz)
                nc.vector.match_replace(
                    out=xz, in_to_replace=m2, in_values=xz, imm_value=NEG)

                # M = sigmoid(X) - sigmoid(X_zapped), fp16 (the
                # non-selected entries are identical fp16 values in both
                # sigmoids and cancel exactly); the 1/(SA*SB) rescale
                # rides the ACT input scale; denom via the activation
                # accumulators.
                s_all = work.tile([P, NR], F16, tag="s_all")
                acc_all = work.tile([P, 1], F32, tag="acc_all")
                nc.scalar.activation(
                    s_all, xs, AF.Sigmoid, scale=ISCALE, accum_out=acc_all)
                s_exc = work.tile([P, NR], F16, tag="s_exc")
                acc_exc = work.tile([P, 1], F32, tag="acc_exc")
                nc.scalar.activation(
                    s_exc, xz, AF.Sigmoid, scale=ISCALE, accum_out=acc_exc)
                mf = comb.tile([P, NR], F16, tag="mf")
                nc.vector.tensor_sub(mf, s_all, s_exc)
                den = work.tile([P, 1], F32, tag="den")
                nc.vector.tensor_sub(den, acc_all, acc_exc)
                rec = comb.tile([P, 1], F32, tag="rec")
                nc.vector.reciprocal(rec, den)
                return mf, rec

            def combine_phase(mm, mf, rec):
                # M^T via one xbar DMA: out[p, c, j] = in[j, c*P+p].
                mt = comb.tile([P, NRC * P], F16, tag="mt")
                nc.sync.dma_start_transpose(
                    mt[:].rearrange("p (c j) -> p c j", c=NRC), mf)
                op = pso.tile([P, D], F32, tag="pso")
                for c in range(NRC):
                    nc.tensor.matmul(
                        op,
                        lhsT=mt[:, c * P:(c + 1) * P],
                        rhs=r1_sb[:, c * D:(c + 1) * D],
                        start=(c == 0),
                        stop=(c == NRC - 1),
                    )
                ot = work.tile([P, D], F32, tag="ot")
                nc.scalar.activation(ot, op, AF.Copy, scale=rec)
                # Output stores on gpsimd: they trail the combine and must
                # not block the sync ring's stacked-tile prefetches.
                nc.gpsimd.dma_start(
                    out=out[mm * P:(mm + 1) * P, :], in_=ot)

            # Software pipeline: gating tile 0 first (PE starts as soon as
            # its operands stream in), then the full local R1 under it;
            # tile m's combine runs after tile m+PEND-1's gating so the PE
            # never waits on the serial DVE top-k chain.
            pending = [(0, *gating_phase(0))]

            # Full local R1 = relu(R @ W^T + b) on every core — no
            # collective (a cross-core AllGather measured ~90us of latency
            # and stalled the in-order PE queue). The lhsT operand is the
            # already-loaded scaled rel_hi chunks (256*R)^T; the 1/SB
            # rescale rides the ReLU's input scale, so the bias matmul
            # adds SB*b (pre-scaled host-side).
            for c in range(NRC):
                k0 = c * P
                pr = pso.tile([P, D], F32, tag="pso")
                for k in range(KC):
                    nc.tensor.matmul(
                        pr,
                        lhsT=bh_k[k][:, k0:k0 + P],
                        rhs=wt_sb[:, k * D:(k + 1) * D],
                        start=(k == 0),
                        stop=False,
                    )
                nc.tensor.matmul(
                    pr, lhsT=ones1, rhs=b_sb, start=False, stop=True)
                nc.scalar.activation(
                    r1_sb[:, c * D:(c + 1) * D], pr, AF.Relu,
                    scale=1.0 / SB)

            for m in range(1, RT):
                pending.append((m, *gating_phase(m)))
                if len(pending) >= PEND:
                    combine_phase(*pending.pop(0))
            while pending:
                combine_phase(*pending.pop(0))

    nc.finalize()
    return nc


def _get_nc():
    global _CACHED
    if _CACHED is None:
        _CACHED = _build()
    return _CACHED


def _split16(x):
    hi = x.astype(np.float16)
    lo = (x - hi.astype(np.float32)).astype(np.float16)
    return hi, lo


def _chunk_part(x):
    """[TWO_D, N] -> [P, KC*N]: feature-chunk k at cols [k*N, (k+1)*N)."""
    n = x.shape[1]
    return np.ascontiguousarray(
        x.reshape(KC, P, n).transpose(1, 0, 2).reshape(P, KC * n))


def _make_in_maps(e1, e2, rel_emb, W_fcs, b_fcs):
    e1 = np.asarray(e1, dtype=np.float32)
    e2 = np.asarray(e2, dtype=np.float32)
    rel_emb = np.asarray(rel_emb, dtype=np.float32)
    W_fcs = np.asarray(W_fcs, dtype=np.float32)
    b_fcs = np.asarray(b_fcs, dtype=np.float32).reshape(1, D)

    stacked = np.concatenate([e1, e2], axis=1) * SA   # [B, 2D]
    a_hi, a_lo = _split16(stacked)
    relT = np.ascontiguousarray((rel_emb * SB).T)     # [2D, NR]
    r_hi, r_lo = _split16(relT)

    # A tiles: [RT*P, TWO_D] with A[m*P+p, k*P+j] = stacked[m*P+j, k*P+p]
    def a_tiles(a):
        return np.ascontiguousarray(
            a.reshape(RT, P, KC, P).transpose(0, 3, 2, 1).reshape(RT * P, TWO_D))

    wkm = _chunk_part(np.ascontiguousarray(W_fcs.T)).astype(np.float16)
    return [
        {
            "A_hi": a_tiles(a_hi[c * BC:(c + 1) * BC]),
            "A_lo": a_tiles(a_lo[c * BC:(c + 1) * BC]),
            "relT_hi": r_hi,
            "relT_lo": r_lo,
            "W_k": wkm,
            # the R1 bias matmul adds SB*b (rescaled away inside the ReLU)
            "b_fcs": (b_fcs * SB).astype(np.float16),
        }
        for c in range(N_CORES)
    ]


def kernel(e1, e2, rel_emb, W_fcs, b_fcs, **_ignored):
    nc = _get_nc()
    in_maps = _make_in_maps(e1, e2, rel_emb, W_fcs, b_fcs)
    res = run_bass_kernel_spmd(nc, in_maps, list(range(N_CORES)))
    return np.concatenate(
        [res.results[c]["out"] for c in range(N_CORES)], axis=0)


# revision 17
# speedup vs baseline: 1.4311x; 1.0258x over previous
"""Trainium2 Bass kernel for the ConvE-style MoE-routing block.

Computes, for each batch row b:
    X = [e1|e2] @ rel_emb.T            # [B, NR] gating logits
    S, idx = top_k(sigmoid(X), 16)
    R1 = relu(rel_emb @ W_fcs.T + b)   # [NR, D]
    out = sum_k S_k * R1[idx_k] / sum_k S_k

Reformulated gather-free: zap the top-16 logits per row with two
(max8 + match_replace) rounds, then M = sigmoid(X) - sigmoid(X_zapped)
is exactly the top-16 sigmoid weights (0 elsewhere), so
    out = (M @ R1) / rowsum(M)
runs on the tensor engine as a dense matmul.

Precision: fp32 matmul costs 4 PE cycles/row and f32r (1 cycle/row)
truncates inputs to ~bf16, which flips ~25% of rows' top-16 sets. The
gating here instead uses a 3-term fp16 split at full PE rate:
    X*SA*SB = a_hi@b_hi + a_lo@b_hi + a_hi@b_lo
with a = stacked*SA, b = rel*SB pre-split hi/lo into fp16 host-side.
The scales keep every split value in fp16 normal range (subnormal-FTZ
safe); residual error ~2e-7 abs vs top-16 gap scale ~1.5e-2, giving 0
selection flips on N(0,1)-type data (validated offline vs fp64).
Sigmoids fold the 1/(SA*SB) rescale into the ACT input scale. R1 and
the combine matmul are fp16 (value-grade).

Layouts: the PE contracts along partitions, so all contraction
operands are marshalled host-side in numpy (pure input marshalling, no
FLOPs) into SBUF-native [128, free] layouts and DMA'd with one
descriptor each; the kernel spends no engine time on transposes except
M^T (data-dependent, via DMA xbar).

Data-parallel over batch across 8 cores; rel/W replicated; R1 computed
fully locally on every core straight from the already-loaded scaled
rel_hi chunks (an AllGather of a sharded R1 measured ~90us of
collective latency and stalled the in-order PE queue — recomputing is
144 matmuls ~= 38us and makes the cores fully independent). Combines
trail gating by PEND tiles so the PE never waits on the serial DVE
top-k chain. DMA traffic is spread over the three DMA-capable queues
(sync/scalar/gpsimd); output stores ride gpsimd so they never block
the stacked-tile prefetches.
"""
import numpy as np

import concourse.bacc as bacc
import concourse.mybir as mybir
from concourse.bass_utils import run_bass_kernel_spmd
from concourse.tile import TileContext

P = 128
D = 512
TWO_D = 1024
NR = 2048
B = 8192
N_CORES = 8
BC = B // N_CORES      # 1024 batch rows per core
RT = BC // P           # 8 row tiles per core
KC = TWO_D // P        # 8 feature (contraction) chunks
NRC = NR // P          # 16 rel chunks
NLOC = NRC // N_CORES  # rel chunks per core for sharded R1
PEND = 5               # combine pipeline depth: the deferred combines
                       # pack the pipeline drain behind the last tile's
                       # serial DVE top-k chain

SA = 64.0              # stacked pre-scale (fp16-normal-range splits)
SB = 256.0             # rel pre-scale
ISCALE = 1.0 / (SA * SB)
NEG = -1.1e6           # sigmoid(NEG*ISCALE) ~ 8e-30: cancels exactly

F32 = mybir.dt.float32
F16 = mybir.dt.float16
AF = mybir.ActivationFunctionType

_CACHED = None


def _build():
    nc = bacc.Bacc("TRN2", target_bir_lowering=False, debug=True)
    # Host-marshalled operand layouts (see module docstring).
    a_hi_d = nc.declare_dram_parameter("A_hi", [RT * P, TWO_D], F16, isOutput=False)
    a_lo_d = nc.declare_dram_parameter("A_lo", [RT * P, TWO_D], F16, isOutput=False)
    rH = nc.declare_dram_parameter("relT_hi", [TWO_D, NR], F16, isOutput=False)
    rL = nc.declare_dram_parameter("relT_lo", [TWO_D, NR], F16, isOutput=False)
    wk = nc.declare_dram_parameter("W_k", [P, KC * D], F16, isOutput=False)
    bf = nc.declare_dram_parameter("b_fcs", [1, D], F16, isOutput=False)
    out = nc.declare_dram_parameter("out", [BC, D], F32, isOutput=True)

    with TileContext(nc) as tc:
        with (
            tc.tile_pool(name="consts", bufs=1) as consts,
            tc.tile_pool(name="persist", bufs=1) as persist,
            tc.tile_pool(name="psx", bufs=3, space="PSUM") as psx,
            tc.tile_pool(name="pso", bufs=2, space="PSUM") as pso,
            tc.tile_pool(name="work", bufs=2) as work,
            tc.tile_pool(name="comb", bufs=PEND + 1) as comb,
        ):
            ones1_f32 = consts.tile([1, P], F32)
            nc.vector.memset(ones1_f32, 1.0)
            ones1 = consts.tile([1, P], F16)
            nc.vector.tensor_copy(ones1, ones1_f32)

            # Tile-0 stacked splits lead the sync ring: the PE's first
            # gating matmul depends only on these + bh_0.
            a_tiles = {}

            def load_a(m):
                ah = work.tile([P, TWO_D], F16, tag="ah")
                nc.sync.dma_start(out=ah, in_=a_hi_d[m * P:(m + 1) * P, :])
                al = work.tile([P, TWO_D], F16, tag="al")
                nc.sync.dma_start(out=al, in_=a_lo_d[m * P:(m + 1) * P, :])
                a_tiles[m] = (ah, al)

            load_a(0)

            # Gating rel chunks: b_hi next on the sync ring; b_lo split
            # over the scalar/gpsimd rings (needed only from the third
            # gating pass of each tile).
            bh_k = []
            bl_k = []
            for k in range(KC):
                t = persist.tile([P, NR], F16, tag=f"bh{k}")
                nc.sync.dma_start(out=t, in_=rH[k * P:(k + 1) * P, :])
                bh_k.append(t)
            # rel_lo split over the scalar/gpsimd rings (chunk k is needed
            # ~3us after bh_k under the per-k term interleave); the R1
            # operands follow (R1 runs only after gating tile 0).
            for k in range(KC):
                t = persist.tile([P, NR], F16, tag=f"bl{k}")
                (nc.scalar if k < KC // 2 else nc.gpsimd).dma_start(
                    out=t, in_=rL[k * P:(k + 1) * P, :])
                bl_k.append(t)
            wt_sb = persist.tile([P, KC * D], F16)
            nc.scalar.dma_start(out=wt_sb, in_=wk[:])
            b_sb = consts.tile([1, D], F16)
            nc.scalar.dma_start(out=b_sb, in_=bf[:])

            # R1: rel-chunk c at cols [c*D, (c+1)*D), fp16 (value-grade).
            r1_sb = persist.tile([P, NRC * D], F16)

            def gating_phase(m):
                if m + 1 < RT:
                    load_a(m + 1)   # prefetch behind this tile's compute
                ah, al = a_tiles.pop(m)

                # Gating X*SA*SB via the 3-term fp16 split, fp32 PSUM
                # accumulation (24 matmuls per 512-col bank). k-outer so
                # each rel chunk pair (bh_k, bl_k) is fully consumed as it
                # streams in — the aggregate input-DMA rate, not any single
                # chunk, gates the head of the pipeline.
                xs = work.tile([P, NR], F32, tag="xs")
                xp0 = psx.tile([P, TWO_D], F32, tag="xph")
                xp1 = psx.tile([P, TWO_D], F32, tag="xph")
                xp = (xp0, xp1)
                for k in range(KC):
                    for t, (A, Bk) in enumerate(
                            ((ah, bh_k), (al, bh_k), (ah, bl_k))):
                        lhs = A[:, k * P:(k + 1) * P]
                        for hb in range(2):
                            for nb in range(2):
                                nc.tensor.matmul(
                                    xp[hb][:, nb * 512:(nb + 1) * 512],
                                    lhsT=lhs,
                                    rhs=Bk[k][:, (hb * 2 + nb) * 512:
                                              (hb * 2 + nb + 1) * 512],
                                    start=(t == 0 and k == 0),
                                    stop=(t == 2 and k == KC - 1),
                                )
                for q in range(4):
                    nc.scalar.activation(
                        xs[:, q * 512:(q + 1) * 512],
                        xp[q // 2][:, (q % 2) * 512:(q % 2 + 1) * 512],
                        AF.Copy)

                # Zap top-16 values (fp32 scan, selection-grade).
                m1 = work.tile([P, 8], F32, tag="m1")
                nc.vector.max(out=m1, in_=xs)
                xz = work.tile([P, NR], F32, tag="xz")
                nc.vector.match_replace(
                    out=xz, in_to_replace=m1, in_values=xs, imm_value=NEG)
                m2 = work.tile([P, 8], F32, tag="m2")
                nc.vector.max(out=m2, in_=xz)
                nc.vector.match_replace(
                    out=xz, in_to_replace=m2, in_values=xz, imm_value=NEG)

                # M = sigmoid(X) - sigmoid(X_zapped), fp16 (the
                # non-selected entries are identical fp16 values in both
                # sigmoids and cancel exactly); the 1/(SA*SB) rescale
                # rides the ACT input scale; denom via the activation
                # accumulators.
                s_all = work.tile([P, NR], F16, tag="s_all")
                acc_all = work.tile([P, 1], F32, tag="acc_all")
                nc.scalar.activation(
                    s_all, xs, AF.Sigmoid, scale=ISCALE, accum_out=acc_all)
                s_exc = work.tile([P, NR], F16, tag="s_exc")
                acc_exc = work.tile([P, 1], F32, tag="acc_exc")
                nc.scalar.activation(
                    s_exc, xz, AF.Sigmoid, scale=ISCALE, accum_out=acc_exc)
                mf = comb.tile([P, NR], F16, tag="mf")
                nc.vector.tensor_sub(mf, s_all, s_exc)
                den = work.tile([P, 1], F32, tag="den")
                nc.vector.tensor_sub(den, acc_all, acc_exc)
                rec = comb.tile([P, 1], F32, tag="rec")
                nc.vector.reciprocal(rec, den)
                return mf, rec

            def combine_phase(mm, mf, rec):
                # M^T via one xbar DMA: out[p, c, j] = in[j, c*P+p].
                mt = comb.tile([P, NRC * P], F16, tag="mt")
                nc.sync.dma_start_transpose(
                    mt[:].rearrange("p (c j) -> p c j", c=NRC), mf)
                op = pso.tile([P, D], F32, tag="pso")
                for c in range(NRC):
                    nc.tensor.matmul(
                        op,
                        lhsT=mt[:, c * P:(c + 1) * P],
                        rhs=r1_sb[:, c * D:(c + 1) * D],
                        start=(c == 0),
                        stop=(c == NRC - 1),
                    )
                ot = work.tile([P, D], F32, tag="ot")
                nc.scalar.activation(ot, op, AF.Copy, scale=rec)
                # Output stores on gpsimd: they trail the combine and must
                # not block the sync ring's stacked-tile prefetches.
                nc.gpsimd.dma_start(
                    out=out[mm * P:(mm + 1) * P, :], in_=ot)

            # Software pipeline: gating tile 0 first (PE starts as soon as
            # its operands stream in), then the full local R1 under it;
            # tile m's combine runs after tile m+PEND-1's gating so the PE
            # never waits on the serial DVE top-k chain.
            pending = [(0, *gating_phase(0))]

            # Full local R1 = relu(R @ W^T + b) on every core — no
            # collective (a cross-core AllGather measured ~90us of latency
            # and stalled the in-order PE queue). The lhsT operand is the
            # already-loaded scaled rel_hi chunks (256*R)^T; the 1/SB
            # rescale rides the ReLU's input scale, so the bias matmul
            # adds SB*b (pre-scaled host-side).
            for c in range(NRC):
                k0 = c * P
                pr = pso.tile([P, D], F32, tag="pso")
                for k in range(KC):
                    nc.tensor.matmul(
                        pr,
                        lhsT=bh_k[k][:, k0:k0 + P],
                        rhs=wt_sb[:, k * D:(k + 1) * D],
                        start=(k == 0),
                        stop=False,
                    )
                nc.tensor.matmul(
                    pr, lhsT=ones1, rhs=b_sb, start=False, stop=True)
                nc.scalar.activation(
                    r1_sb[:, c * D:(c + 1) * D], pr, AF.Relu,
                    scale=1.0 / SB)

            for m in range(1, RT):
                pending.append((m, *gating_phase(m)))
                if len(pending) >= PEND:
                    combine_phase(*pending.pop(0))
            while pending:
                combine_phase(*pending.pop(0))

    nc.finalize()
    return nc


def _get_nc():
    global _CACHED
    if _CACHED is None:
        _CACHED = _build()
    return _CACHED


def _split16(x):
    hi = x.astype(np.float16)
    lo = (x - hi.astype(np.float32)).astype(np.float16)
    return hi, lo


def _chunk_part(x):
    """[TWO_D, N] -> [P, KC*N]: feature-chunk k at cols [k*N, (k+1)*N)."""
    n = x.shape[1]
    return np.ascontiguousarray(
        x.reshape(KC, P, n).transpose(1, 0, 2).reshape(P, KC * n))


def _make_in_maps(e1, e2, rel_emb, W_fcs, b_fcs):
    e1 = np.asarray(e1, dtype=np.float32)
    e2 = np.asarray(e2, dtype=np.float32)
    rel_emb = np.asarray(rel_emb, dtype=np.float32)
    W_fcs = np.asarray(W_fcs, dtype=np.float32)
    b_fcs = np.asarray(b_fcs, dtype=np.float32).reshape(1, D)

    stacked = np.concatenate([e1, e2], axis=1) * SA   # [B, 2D]
    a_hi, a_lo = _split16(stacked)
    relT = np.ascontiguousarray((rel_emb * SB).T)     # [2D, NR]
    r_hi, r_lo = _split16(relT)

    # A tiles: [RT*P, TWO_D] with A[m*P+p, k*P+j] = stacked[m*P+j, k*P+p]
    def a_tiles(a):
        return np.ascontiguousarray(
            a.reshape(RT, P, KC, P).transpose(0, 3, 2, 1).reshape(RT * P, TWO_D))

    wkm = _chunk_part(np.ascontiguousarray(W_fcs.T)).astype(np.float16)
    return [
        {
            "A_hi": a_tiles(a_hi[c * BC:(c + 1) * BC]),
            "A_lo": a_tiles(a_lo[c * BC:(c + 1) * BC]),
            "relT_hi": r_hi,
            "relT_lo": r_lo,
            "W_k": wkm,
            # the R1 bias matmul adds SB*b (rescaled away inside the ReLU)
            "b_fcs": (b_fcs * SB).astype(np.float16),
        }
        for c in range(N_CORES)
    ]


def kernel(e1, e2, rel_emb, W_fcs, b_fcs, **_ignored):
    nc = _get_nc()
    in_maps = _make_in_maps(e1, e2, rel_emb, W_fcs, b_fcs)
    res = run_bass_kernel_spmd(nc, in_maps, list(range(N_CORES)))
    return np.concatenate(
        [res.results[c]["out"] for c in range(N_CORES)], axis=0)


# revision 21
# speedup vs baseline: 1.4697x; 1.0270x over previous
"""Trainium2 Bass kernel for the ConvE-style MoE-routing block.

Computes, for each batch row b:
    X = [e1|e2] @ rel_emb.T            # [B, NR] gating logits
    S, idx = top_k(sigmoid(X), 16)
    R1 = relu(rel_emb @ W_fcs.T + b)   # [NR, D]
    out = sum_k S_k * R1[idx_k] / sum_k S_k

Reformulated gather-free: zap the top-16 logits per row with two
(max8 + match_replace) rounds, then M = sigmoid(X) - sigmoid(X_zapped)
is exactly the top-16 sigmoid weights (0 elsewhere), so
    out = (M @ R1) / rowsum(M)
runs on the tensor engine as a dense matmul.

Precision: fp32 matmul costs 4 PE cycles/row and f32r (1 cycle/row)
truncates inputs to ~bf16, which flips ~25% of rows' top-16 sets. The
gating here instead uses a 3-term fp16 split at full PE rate:
    X*SA*SB = a_hi@b_hi + a_lo@b_hi + a_hi@b_lo
with a = stacked*SA, b = rel*SB pre-split hi/lo into fp16 host-side.
The scales keep every split value in fp16 normal range (subnormal-FTZ
safe); residual error ~2e-7 abs vs top-16 gap scale ~1.5e-2, giving 0
selection flips on N(0,1)-type data (validated offline vs fp64).
Sigmoids fold the 1/(SA*SB) rescale into the ACT input scale. R1 and
the combine matmul are fp16 (value-grade).

Layouts: the PE contracts along partitions, so all contraction
operands are marshalled host-side in numpy (pure input marshalling, no
FLOPs) into SBUF-native [128, free] layouts and DMA'd with one
descriptor each; the kernel spends no engine time on transposes except
M^T (data-dependent, via DMA xbar).

Data-parallel over batch across 8 cores; rel/W replicated; R1 computed
fully locally on every core straight from the already-loaded scaled
rel_hi chunks (an AllGather of a sharded R1 measured ~90us of
collective latency and stalled the in-order PE queue — recomputing is
144 matmuls ~= 38us and makes the cores fully independent). Combines
trail gating by PEND tiles so the PE never waits on the serial DVE
top-k chain. DMA traffic is spread over the three DMA-capable queues
(sync/scalar/gpsimd); output stores ride gpsimd so they never block
the stacked-tile prefetches.
"""
import numpy as np

import concourse.bacc as bacc
import concourse.mybir as mybir
from concourse.bass_utils import run_bass_kernel_spmd
from concourse.tile import TileContext

P = 128
D = 512
TWO_D = 1024
NR = 2048
B = 8192
N_CORES = 8
BC = B // N_CORES      # 1024 batch rows per core
RT = BC // P           # 8 row tiles per core
KC = TWO_D // P        # 8 feature (contraction) chunks
NRC = NR // P          # 16 rel chunks
NLOC = NRC // N_CORES  # rel chunks per core for sharded R1
PEND = 6               # combine pipeline depth: the deferred combines
                       # pack the pipeline drain behind the last tile's
                       # serial DVE top-k chain

SA = 64.0              # stacked pre-scale (fp16-normal-range splits)
SB = 256.0             # rel pre-scale
ISCALE = 1.0 / (SA * SB)
NEG = -1.1e6           # sigmoid(NEG*ISCALE) ~ 8e-30: cancels exactly

F32 = mybir.dt.float32
F16 = mybir.dt.float16
AF = mybir.ActivationFunctionType

_CACHED = None


def _build():
    nc = bacc.Bacc("TRN2", target_bir_lowering=False, debug=True)
    # Host-marshalled operand layouts (see module docstring).
    a_hi_d = nc.declare_dram_parameter("A_hi", [RT * P, TWO_D], F16, isOutput=False)
    a_lo_d = nc.declare_dram_parameter("A_lo", [RT * P, TWO_D], F16, isOutput=False)
    rH = nc.declare_dram_parameter("relT_hi", [TWO_D, NR], F16, isOutput=False)
    rL = nc.declare_dram_parameter("relT_lo", [TWO_D, NR], F16, isOutput=False)
    wk = nc.declare_dram_parameter("W_k", [P, KC * D], F16, isOutput=False)
    bf = nc.declare_dram_parameter("b_fcs", [1, D], F16, isOutput=False)
    out = nc.declare_dram_parameter("out", [BC, D], F32, isOutput=True)

    with TileContext(nc) as tc:
        with (
            tc.tile_pool(name="consts", bufs=1) as consts,
            tc.tile_pool(name="persist", bufs=1) as persist,
            tc.tile_pool(name="psx", bufs=3, space="PSUM") as psx,
            tc.tile_pool(name="pso", bufs=2, space="PSUM") as pso,
            tc.tile_pool(name="work", bufs=2) as work,
            tc.tile_pool(name="comb", bufs=PEND + 1) as comb,
            # combines serialize on the PE, so M^T staging only needs a
            # short pipeline regardless of PEND
            tc.tile_pool(name="combt", bufs=3) as combt,
        ):
            ones1_f32 = consts.tile([1, P], F32)
            nc.vector.memset(ones1_f32, 1.0)
            ones1 = consts.tile([1, P], F16)
            nc.vector.tensor_copy(ones1, ones1_f32)

            # Tile-0 stacked splits lead the sync ring: the PE's first
            # gating matmul depends only on these + bh_0.
            a_tiles = {}

            def load_a(m):
                ah = work.tile([P, TWO_D], F16, tag="ah")
                nc.sync.dma_start(out=ah, in_=a_hi_d[m * P:(m + 1) * P, :])
                al = work.tile([P, TWO_D], F16, tag="al")
                nc.sync.dma_start(out=al, in_=a_lo_d[m * P:(m + 1) * P, :])
                a_tiles[m] = (ah, al)

            load_a(0)

            # Gating rel chunks round-robined across all three DMA rings
            # in need order (pair k feeds gating ~3us after pair k-1): a
            # ring completes its transfers roughly cumulative-bytes /
            # ring-bandwidth, so clustering the early chunks on one ring
            # starves the PE. The R1 operands (wt, b) trail on scalar —
            # R1 runs only after gating tile 0.
            bh_k = [None] * KC
            bl_k = [None] * KC
            rings = (nc.scalar, nc.gpsimd, nc.sync)
            for i in range(2 * KC):
                k, hi = divmod(i, 2)
                src = rH if hi == 0 else rL
                t = persist.tile([P, NR], F16,
                                 tag=f"{'bh' if hi == 0 else 'bl'}{k}")
                rings[i % 3].dma_start(out=t, in_=src[k * P:(k + 1) * P, :])
                (bh_k if hi == 0 else bl_k)[k] = t
            wt_sb = persist.tile([P, KC * D], F16)
            nc.scalar.dma_start(out=wt_sb, in_=wk[:])
            b_sb = consts.tile([1, D], F16)
            nc.scalar.dma_start(out=b_sb, in_=bf[:])

            # R1: rel-chunk c at cols [c*D, (c+1)*D), fp16 (value-grade).
            r1_sb = persist.tile([P, NRC * D], F16)

            def gating_phase(m):
                if m + 1 < RT:
                    load_a(m + 1)   # prefetch behind this tile's compute
                ah, al = a_tiles.pop(m)

                # Gating X*SA*SB via the 3-term fp16 split, fp32 PSUM
                # accumulation (24 matmuls per 512-col bank). k-outer so
                # each rel chunk pair (bh_k, bl_k) is fully consumed as it
                # streams in — the aggregate input-DMA rate, not any single
                # chunk, gates the head of the pipeline.
                xs = work.tile([P, NR], F32, tag="xs")
                xp0 = psx.tile([P, TWO_D], F32, tag="xph")
                xp1 = psx.tile([P, TWO_D], F32, tag="xph")
                xp = (xp0, xp1)
                for k in range(KC):
                    for t, (A, Bk) in enumerate(
                            ((ah, bh_k), (al, bh_k), (ah, bl_k))):
                        lhs = A[:, k * P:(k + 1) * P]
                        for hb in range(2):
                            for nb in range(2):
                                nc.tensor.matmul(
                                    xp[hb][:, nb * 512:(nb + 1) * 512],
                                    lhsT=lhs,
                                    rhs=Bk[k][:, (hb * 2 + nb) * 512:
                                              (hb * 2 + nb + 1) * 512],
                                    start=(t == 0 and k == 0),
                                    stop=(t == 2 and k == KC - 1),
                                )
                for q in range(4):
                    nc.scalar.activation(
                        xs[:, q * 512:(q + 1) * 512],
                        xp[q // 2][:, (q % 2) * 512:(q % 2 + 1) * 512],
                        AF.Copy)

                # Zap top-16 values (fp32 scan, selection-grade).
                m1 = work.tile([P, 8], F32, tag="m1")
                nc.vector.max(out=m1, in_=xs)
                xz = work.tile([P, NR], F32, tag="xz")
                nc.vector.match_replace(
                    out=xz, in_to_replace=m1, in_values=xs, imm_value=NEG)
                m2 = work.tile([P, 8], F32, tag="m2")
                nc.vector.max(out=m2, in_=xz)
                nc.vector.match_replace(
                    out=xz, in_to_replace=m2, in_values=xz, imm_value=NEG)

                # M = sigmoid(X) - sigmoid(X_zapped), fp16 (the
                # non-selected entries are identical fp16 values in both
                # sigmoids and cancel exactly); the 1/(SA*SB) rescale
                # rides the ACT input scale; denom via the activation
                # accumulators.
                s_all = work.tile([P, NR], F16, tag="s_all")
                acc_all = work.tile([P, 1], F32, tag="acc_all")
                nc.scalar.activation(
                    s_all, xs, AF.Sigmoid, scale=ISCALE, accum_out=acc_all)
                s_exc = work.tile([P, NR], F16, tag="s_exc")
                acc_exc = work.tile([P, 1], F32, tag="acc_exc")
                nc.scalar.activation(
                    s_exc, xz, AF.Sigmoid, scale=ISCALE, accum_out=acc_exc)
                mf = comb.tile([P, NR], F16, tag="mf")
                nc.vector.tensor_sub(mf, s_all, s_exc)
                den = work.tile([P, 1], F32, tag="den")
                nc.vector.tensor_sub(den, acc_all, acc_exc)
                rec = comb.tile([P, 1], F32, tag="rec")
                nc.vector.reciprocal(rec, den)
                return mf, rec

            def combine_phase(mm, mf, rec):
                # M^T via one xbar DMA: out[p, c, j] = in[j, c*P+p].
                mt = combt.tile([P, NRC * P], F16, tag="mt")
                nc.sync.dma_start_transpose(
                    mt[:].rearrange("p (c j) -> p c j", c=NRC), mf)
                op = pso.tile([P, D], F32, tag="pso")
                for c in range(NRC):
                    nc.tensor.matmul(
                        op,
                        lhsT=mt[:, c * P:(c + 1) * P],
                        rhs=r1_sb[:, c * D:(c + 1) * D],
                        start=(c == 0),
                        stop=(c == NRC - 1),
                    )
                ot = work.tile([P, D], F32, tag="ot")
                nc.scalar.activation(ot, op, AF.Copy, scale=rec)
                # Output stores on gpsimd: they trail the combine and must
                # not block the sync ring's stacked-tile prefetches.
                nc.gpsimd.dma_start(
                    out=out[mm * P:(mm + 1) * P, :], in_=ot)

            # Software pipeline: gating tile 0 first (PE starts as soon as
            # its operands stream in), then the full local R1 under it;
            # tile m's combine runs after tile m+PEND-1's gating so the PE
            # never waits on the serial DVE top-k chain.
            pending = [(0, *gating_phase(0))]

            # Full local R1 = relu(R @ W^T + b) on every core — no
            # collective (a cross-core AllGather measured ~90us of latency
            # and stalled the in-order PE queue). The lhsT operand is the
            # already-loaded scaled rel_hi chunks (256*R)^T; the 1/SB
            # rescale rides the ReLU's input scale, so the bias matmul
            # adds SB*b (pre-scaled host-side).
            for c in range(NRC):
                k0 = c * P
                pr = pso.tile([P, D], F32, tag="pso")
                for k in range(KC):
                    nc.tensor.matmul(
                        pr,
                        lhsT=bh_k[k][:, k0:k0 + P],
                        rhs=wt_sb[:, k * D:(k + 1) * D],
                        start=(k == 0),
                        stop=False,
                    )
                nc.tensor.matmul(
                    pr, lhsT=ones1, rhs=b_sb, start=False, stop=True)
                nc.scalar.activation(
                    r1_sb[:, c * D:(c + 1) * D], pr, AF.Relu,
                    scale=1.0 / SB)

            for m in range(1, RT):
                pending.append((m, *gating_phase(m)))
                if len(pending) >= PEND:
                    combine_phase(*pending.pop(0))
            while pending:
                combine_phase(*pending.pop(0))

    nc.finalize()
    return nc


def _get_nc():
    global _CACHED
    if _CACHED is None:
        _CACHED = _build()
    return _CACHED


def _split16(x):
    hi = x.astype(np.float16)
    lo = (x - hi.astype(np.float32)).astype(np.float16)
    return hi, lo


def _chunk_part(x):
    """[TWO_D, N] -> [P, KC*N]: feature-chunk k at cols [k*N, (k+1)*N)."""
    n = x.shape[1]
    return np.ascontiguousarray(
        x.reshape(KC, P, n).transpose(1, 0, 2).reshape(P, KC * n))


def _make_in_maps(e1, e2, rel_emb, W_fcs, b_fcs):
    e1 = np.asarray(e1, dtype=np.float32)
    e2 = np.asarray(e2, dtype=np.float32)
    rel_emb = np.asarray(rel_emb, dtype=np.float32)
    W_fcs = np.asarray(W_fcs, dtype=np.float32)
    b_fcs = np.asarray(b_fcs, dtype=np.float32).reshape(1, D)

    stacked = np.concatenate([e1, e2], axis=1) * SA   # [B, 2D]
    a_hi, a_lo = _split16(stacked)
    relT = np.ascontiguousarray((rel_emb * SB).T)     # [2D, NR]
    r_hi, r_lo = _split16(relT)

    # A tiles: [RT*P, TWO_D] with A[m*P+p, k*P+j] = stacked[m*P+j, k*P+p]
    def a_tiles(a):
        return np.ascontiguousarray(
            a.reshape(RT, P, KC, P).transpose(0, 3, 2, 1).reshape(RT * P, TWO_D))

    wkm = _chunk_part(np.ascontiguousarray(W_fcs.T)).astype(np.float16)
    return [
        {
            "A_hi": a_tiles(a_hi[c * BC:(c + 1) * BC]),
            "A_lo": a_tiles(a_lo[c * BC:(c + 1) * BC]),
            "relT_hi": r_hi,
            "relT_lo": r_lo,
            "W_k": wkm,
            # the R1 bias matmul adds SB*b (rescaled away inside the ReLU)
            "b_fcs": (b_fcs * SB).astype(np.float16),
        }
        for c in range(N_CORES)
    ]


def kernel(e1, e2, rel_emb, W_fcs, b_fcs, **_ignored):
    nc = _get_nc()
    in_maps = _make_in_maps(e1, e2, rel_emb, W_fcs, b_fcs)
    res = run_bass_kernel_spmd(nc, in_maps, list(range(N_CORES)))
    return np.concatenate(
        [res.results[c]["out"] for c in range(N_CORES)], axis=0)
